# revision 18
# baseline (speedup 1.0000x reference)
"""Trainium2 Bass kernel for nn_Attention_msa (sparse masked-softmax attention).

Sharding: 8-way over query rows (each core owns 256 query rows for ALL 8
heads). The k/v projections (which need all 2048 keys) are computed
redundantly on every core — this avoids any collective (on-chip collectives
run at ~32 GB/s, so reducing a 16 MB [N,N] attention tensor would dominate).
The head-means of attn and attn_cls_raw (the sim outputs) then become purely
local reductions over the 8 heads each core already holds.

attn_cls_raw's head-mean uses the concat identity:
  mean_h vn_h[i].vn_h[j] = (1/H) * Vn[i].Vn[j],  Vn = concat_h vn_h  (N x C)
so it is a single C-contraction matmul per core instead of an H-reduction.

All matmuls run in bf16 (fp32 PSUM accumulate); masks, logits, exp and all
normalizers stay fp32. Emulated end-to-end numerics: max rel err ~2.5e-3.
"""
import sys

for _p in ("/opt/trn_rl_repo",):
    if _p not in sys.path:
        sys.path.insert(0, _p)

import numpy as np
import ml_dtypes

bf16 = ml_dtypes.bfloat16

import concourse.bacc as bacc
import concourse.bass as bass
import concourse.mybir as mybir
import concourse.tile as tile
from concourse import bass_utils
from concourse.masks import make_identity

F32 = mybir.dt.float32
BF16 = mybir.dt.bfloat16
AF = mybir.ActivationFunctionType
ALU = mybir.AluOpType

# Problem constants (hardcoded per harness contract).
N = 2048          # tokens
C = 1024          # channels
H = 8             # heads
D = 128           # head dim
P = 128           # SBUF partitions
NCORES = 8
R = N // NCORES   # query rows per core (256)
RB = R // P       # row blocks per core (2)
NJB = N // P      # key blocks (16)
NCB = C // P      # contraction chunks (8)
FW = 512          # matmul free-dim chunk
SH = FW // P      # head segments per free chunk (4)
HQ = N // 2       # half of N (psum S-tile width)
SCALE = 25.0


def _seg_bcast(ap_2d, seg, width):
    """(P, seg) tile -> broadcast AP (P, seg, width) with stride-0 inner dim."""
    return bass.AP(tensor=ap_2d.tensor, offset=ap_2d.offset,
                   ap=[list(ap_2d.ap[0]), [1, seg], [0, width]])


def _part_bcast(dram_ap, parts):
    """DRAM row (n,) -> AP (parts, n) broadcast across partitions."""
    return bass.AP(tensor=dram_ap.tensor, offset=dram_ap.offset,
                   ap=[[0, parts]] + [list(a) for a in dram_ap.ap])


def _col_view(dram_ap, ncols):
    """DRAM vector (ncols*128,) -> AP (128, ncols): tile[p, c] = v[c*128+p]."""
    return bass.AP(tensor=dram_ap.tensor, offset=dram_ap.offset,
                   ap=[[1, P], [P, ncols]])


def build_nc():
    import os
    from contextlib import ExitStack

    nc = bacc.Bacc("TRN2", target_bir_lowering=False, debug=False,
                   num_devices=NCORES)

    # ---- per-core DRAM I/O (identical program on every core; all
    # core-dependence flows through the input values) ----
    xT_cls_d = nc.dram_tensor("xT_cls", [C, N], BF16, kind="ExternalInput")
    xT_reg_d = nc.dram_tensor("xT_reg", [C, N], BF16, kind="ExternalInput")
    WT_cls_d = nc.dram_tensor("WT_cls", [C, 3 * C], BF16, kind="ExternalInput")
    WT_reg_d = nc.dram_tensor("WT_reg", [C, 2 * C], BF16, kind="ExternalInput")
    s_cls_d = nc.dram_tensor("s_cls", [N], F32, kind="ExternalInput")
    s_fg_d = nc.dram_tensor("s_fg", [N], F32, kind="ExternalInput")
    xTq_cls_d = nc.dram_tensor("xTq_cls", [C, R], BF16, kind="ExternalInput")
    xTq_reg_d = nc.dram_tensor("xTq_reg", [C, R], BF16, kind="ExternalInput")
    # srow_*_m = score[rows] - 0.1 (host-computed in fp32, exact)
    srow_cls_d = nc.dram_tensor("srow_cls_m", [R], F32, kind="ExternalInput")
    srow_fg_d = nc.dram_tensor("srow_fg_m", [R], F32, kind="ExternalInput")

    x_out_d = nc.dram_tensor("x_slice", [R, 2 * C], F32, kind="ExternalOutput")
    sim_out_d = nc.dram_tensor("sim_slice", [R, N], F32, kind="ExternalOutput")

    with tile.TileContext(nc) as tc, ExitStack() as ctx:
        pp = ctx.enter_context(tc.tile_pool(name="pp", bufs=1))

        # ---------- persistent tiles (alive to the end) ----------
        khatT_c = pp.tile([P, H, N], BF16, name="khatT_c")
        v_nd = [pp.tile([P, C], BF16, name=f"v_nd{jb}") for jb in range(NJB)]
        qhatT_c = pp.tile([P, H, R], BF16, name="qhatT_c")
        qhatT_r = pp.tile([P, H, R], BF16, name="qhatT_r")
        invq_c = [pp.tile([P, H], F32, name=f"invq_c{ib}") for ib in range(RB)]
        invq_r = [pp.tile([P, H], F32, name=f"invq_r{ib}") for ib in range(RB)]
        m_sim = [pp.tile([P, N], BF16, name=f"m_sim{ib}") for ib in range(RB)]
        sim_acc = [pp.tile([P, N], F32, name=f"sim_acc{ib}") for ib in range(RB)]
        x_acc = [pp.tile([P, C], F32, name=f"x_acc{ib}") for ib in range(RB)]
        ident = pp.tile([P, P], BF16, name="ident")
        scol_cls = pp.tile([P, NJB], F32, name="scol_cls")
        scol_fg = pp.tile([P, NJB], F32, name="scol_fg")
        srow_c = pp.tile([P, RB], F32, name="srow_c")
        srow_f = pp.tile([P, RB], F32, name="srow_f")

        # ================= shared projection machinery =================
        def kv_proj(xT_tiles, w_dram, w_col0, scol, dst_big, is_v,
                    sb, ps, wp, wtag):
            """All-token projection of one C-wide output group, streamed in
            512-wide output chunks (fc), 128-token blocks (jb)."""
            for fc in range(C // FW):
                w_tiles = []
                for cc in range(NCB):
                    wt = wp.tile([P, FW], BF16, tag=f"{wtag}{cc}",
                                 name=f"{wtag}_{cc}")
                    c0 = w_col0 + fc * FW
                    nc.sync.dma_start(
                        out=wt,
                        in_=w_dram.ap()[cc * P:(cc + 1) * P, c0:c0 + FW])
                    w_tiles.append(wt)
                for jb in range(NJB):
                    kps = ps.tile([P, FW], F32, tag="kps", name=f"kps{jb}")
                    for cc in range(NCB):
                        nc.tensor.matmul(
                            kps, xT_tiles[cc][:, jb * P:(jb + 1) * P],
                            w_tiles[cc], start=(cc == 0),
                            stop=(cc == NCB - 1))
                    if is_v:
                        nc.vector.tensor_copy(
                            v_nd[jb][:, fc * FW:(fc + 1) * FW], kps)
                    ksq = sb.tile([P, FW], BF16, tag="ksq", name=f"ksq{jb}")
                    nc.scalar.activation(ksq, kps, AF.Square)
                    ss = sb.tile([P, SH], F32, tag="ss", name=f"ss{jb}")
                    nc.vector.tensor_reduce(
                        out=ss, in_=ksq.rearrange("p (h e) -> p h e", h=SH),
                        op=ALU.add, axis=mybir.AxisListType.X)
                    nrm = sb.tile([P, SH], F32, tag="nrm", name=f"nrm{jb}")
                    nc.scalar.activation(nrm, ss, AF.Sqrt)
                    inv = sb.tile([P, SH], F32, tag="inv", name=f"inv{jb}")
                    nc.vector.reciprocal(inv, nrm)
                    if not is_v:
                        nc.vector.tensor_scalar(inv, inv, scol[:, jb:jb + 1],
                                                SCALE, ALU.mult, ALU.mult)
                    khat = sb.tile([P, FW], BF16, tag="khat", name=f"khat{jb}")
                    nc.vector.tensor_tensor(
                        out=khat.rearrange("p (h e) -> p h e", h=SH),
                        in0=kps.rearrange("p (h e) -> p h e", h=SH),
                        in1=_seg_bcast(inv, SH, P), op=ALU.mult)
                    tps = ps.tile([P, SH, P], BF16, tag="tps", name=f"tps{jb}")
                    for hh in range(SH):
                        nc.tensor.transpose(tps[:, hh, :],
                                            khat[:, hh * P:(hh + 1) * P],
                                            ident)
                    nc.vector.tensor_copy(
                        dst_big[:, fc * SH:(fc + 1) * SH,
                                jb * P:(jb + 1) * P],
                        tps)

        def q_proj(xTq_tiles, w_dram, w_col0, dst_big, invq, sb, ps, wp,
                   wtag, xori_cb=None, vnqT=None):
            for fc in range(C // FW):
                w_tiles = []
                for cc in range(NCB):
                    wt = wp.tile([P, FW], BF16, tag=f"{wtag}{cc}",
                                 name=f"{wtag}_{cc}")
                    c0 = w_col0 + fc * FW
                    nc.sync.dma_start(
                        out=wt,
                        in_=w_dram.ap()[cc * P:(cc + 1) * P, c0:c0 + FW])
                    w_tiles.append(wt)
                for ib in range(RB):
                    qps = ps.tile([P, FW], F32, tag="kps", name=f"qps{ib}")
                    for cc in range(NCB):
                        nc.tensor.matmul(
                            qps, xTq_tiles[cc][:, ib * P:(ib + 1) * P],
                            w_tiles[cc], start=(cc == 0),
                            stop=(cc == NCB - 1))
                    if xori_cb is not None:
                        xori_cb(qps, fc, ib)
                    qsq = sb.tile([P, FW], BF16, tag="ksq", name=f"qsq{ib}")
                    nc.scalar.activation(qsq, qps, AF.Square)
                    ss = sb.tile([P, SH], F32, tag="ss", name=f"qss{ib}")
                    nc.vector.tensor_reduce(
                        out=ss, in_=qsq.rearrange("p (h e) -> p h e", h=SH),
                        op=ALU.add, axis=mybir.AxisListType.X)
                    nrm = sb.tile([P, SH], F32, tag="nrm", name=f"qnrm{ib}")
                    nc.scalar.activation(nrm, ss, AF.Sqrt)
                    hs = slice(fc * SH, (fc + 1) * SH)
                    if vnqT is None:
                        nc.vector.reciprocal(invq[ib][:, hs], nrm)
                        qnd = sb.tile([P, FW], BF16, tag="khat",
                                      name=f"qnd{ib}")
                        nc.vector.tensor_copy(qnd, qps)
                    else:
                        inv = sb.tile([P, SH], F32, tag="inv", name=f"oi{ib}")
                        nc.vector.reciprocal(inv, nrm)
                        qnd = sb.tile([P, FW], BF16, tag="khat",
                                      name=f"ovn{ib}")
                        nc.vector.tensor_tensor(
                            out=qnd.rearrange("p (h e) -> p h e", h=SH),
                            in0=qps.rearrange("p (h e) -> p h e", h=SH),
                            in1=_seg_bcast(inv, SH, P), op=ALU.mult)
                    tps = ps.tile([P, SH, P], BF16, tag="tps",
                                  name=f"qtps{ib}")
                    for hh in range(SH):
                        nc.tensor.transpose(tps[:, hh, :],
                                            qnd[:, hh * P:(hh + 1) * P],
                                            ident)
                    dst = dst_big if vnqT is None else vnqT
                    nc.vector.tensor_copy(dst[:, hs, ib * P:(ib + 1) * P],
                                          tps)

        # ================= phase A: cls side =================
        # One shared pool set spans A1 (khat_cls), A2 (v/vn), A3 (q, x_ori,
        # vnq) so the scheduler can overlap sub-phases freely.
        with tc.tile_pool(name="vnTp", bufs=1) as vnTp, \
             tc.tile_pool(name="asb", bufs=2) as a_sb, \
             tc.tile_pool(name="aw", bufs=2) as a_wp:
            vnT_all = vnTp.tile([P, H, N], BF16, name="vnT_all")
            vnqT = vnTp.tile([P, H, R], BF16, name="vnqT")
            xTq = []
            for cc in range(NCB):
                t = vnTp.tile([P, R], BF16, name=f"xTqc{cc}")
                xTq.append(t)

            with tc.tile_pool(name="xtp", bufs=1) as xtp:
                # xT_cls: first halves of every chunk first, so jb 0-7
                # matmuls can start as early as possible
                xT = [xtp.tile([P, N], BF16, name=f"xTc{cc}")
                      for cc in range(NCB)]
                for cc in range(NCB):
                    nc.sync.dma_start(out=xT[cc][:, 0:HQ],
                                      in_=xT_cls_d.ap()[cc * P:(cc + 1) * P,
                                                        0:HQ])
                make_identity(nc, ident)
                nc.sync.dma_start(out=scol_cls,
                                  in_=_col_view(s_cls_d.ap(), NJB))
                for cc in range(NCB):
                    nc.sync.dma_start(out=xT[cc][:, HQ:N],
                                      in_=xT_cls_d.ap()[cc * P:(cc + 1) * P,
                                                        HQ:N])

                with tc.tile_pool(name="aps", bufs=4, space="PSUM") as a_ps:
                    # A1: khat_cls
                    kv_proj(xT, WT_cls_d, C, scol_cls, khatT_c, False,
                            a_sb, a_ps, a_wp, "w")
                    for cc in range(NCB):
                        nc.sync.dma_start(
                            out=xTq[cc],
                            in_=xTq_cls_d.ap()[cc * P:(cc + 1) * P, :])
                    nc.sync.dma_start(out=scol_fg,
                                      in_=_col_view(s_fg_d.ap(), NJB))
                    nc.sync.dma_start(out=srow_c,
                                      in_=_col_view(srow_cls_d.ap(), RB))
                    nc.sync.dma_start(out=srow_f,
                                      in_=_col_view(srow_fg_d.ap(), RB))
                    # A2: v (+ vn -> VnT)
                    kv_proj(xT, WT_cls_d, 2 * C, None, vnT_all, True,
                            a_sb, a_ps, a_wp, "w")
                    for ib in range(RB):
                        nc.gpsimd.memset(sim_acc[ib], 0.0)

                    # A3: q_cls, then x_ori + vnq (row-slice v)
                    q_proj(xTq, WT_cls_d, 0, qhatT_c, invq_c,
                           a_sb, a_ps, a_wp, "w")

                    def xori_cb(qps, fc, ib):
                        xo = a_sb.tile([P, FW], F32, tag="xori", bufs=1,
                                       name=f"xo{ib}")
                        nc.scalar.activation(xo, qps, AF.Copy)
                        nc.sync.dma_start(
                            out=x_out_d.ap()[ib * P:(ib + 1) * P,
                                             C + fc * FW:C + (fc + 1) * FW],
                            in_=xo)

                    q_proj(xTq, WT_cls_d, 2 * C, None, None,
                           a_sb, a_ps, a_wp, "w", xori_cb=xori_cb,
                           vnqT=vnqT)

            # A4: raw = Vn_rows @ Vn^T ; sim mask = (sum_h > 6.0)
            # (own psum pool; overlaps nothing psum-wise but xT is freed)
            with tc.tile_pool(name="a4ps", bufs=1, space="PSUM") as rps:
                for ib in range(RB):
                    raw = rps.tile([P, N], F32, tag="raw", name=f"raw{ib}")
                    for fc in range(4):
                        fs = slice(fc * FW, (fc + 1) * FW)
                        for h in range(H):
                            nc.tensor.matmul(
                                raw[:, fs],
                                vnqT[:, h, ib * P:(ib + 1) * P],
                                vnT_all[:, h, fs],
                                start=(h == 0), stop=(h == H - 1))
                    nc.vector.tensor_scalar(m_sim[ib], raw, 6.0, None,
                                            ALU.is_gt)

        # ================= phase B: reg side =================
        khp = ctx.enter_context(tc.tile_pool(name="khp", bufs=1))
        khatT_r = khp.tile([P, H, N], BF16, name="khatT_r")
        with tc.tile_pool(name="xtpr", bufs=1) as xtp2, \
             tc.tile_pool(name="bsb", bufs=2) as b_sb, \
             tc.tile_pool(name="bps", bufs=4, space="PSUM") as b_ps, \
             tc.tile_pool(name="bw", bufs=2) as b_wp:
            xTr = []
            for cc in range(NCB):
                t = xtp2.tile([P, N], BF16, name=f"xTr{cc}")
                nc.sync.dma_start(out=t[:, 0:HQ],
                                  in_=xT_reg_d.ap()[cc * P:(cc + 1) * P, 0:HQ])
                nc.sync.dma_start(out=t[:, HQ:N],
                                  in_=xT_reg_d.ap()[cc * P:(cc + 1) * P,
                                                    HQ:N])
                xTr.append(t)
            xTqr = []
            for cc in range(NCB):
                t = xtp2.tile([P, R], BF16, name=f"xTqr{cc}")
                nc.sync.dma_start(out=t,
                                  in_=xTq_reg_d.ap()[cc * P:(cc + 1) * P, :])
                xTqr.append(t)
            kv_proj(xTr, WT_reg_d, C, scol_fg, khatT_r, False,
                    b_sb, b_ps, b_wp, "w")
            q_proj(xTqr, WT_reg_d, 0, qhatT_r, invq_r,
                   b_sb, b_ps, b_wp, "w")

        # ---------------- masks (built just before the megaloop) ----------
        mp = ctx.enter_context(tc.tile_pool(name="mp", bufs=1))
        m_cls = [mp.tile([P, N], BF16, name=f"m_cls{ib}") for ib in range(RB)]
        m_fg = [mp.tile([P, N], BF16, name=f"m_fg{ib}") for ib in range(RB)]
        with tc.tile_pool(name="pre", bufs=1) as pre:
            sb_c = pre.tile([P, N], F32, name="sb_c")
            sb_f = pre.tile([P, N], F32, name="sb_f")
            nc.gpsimd.dma_start(out=sb_c, in_=_part_bcast(s_cls_d.ap(), P))
            nc.gpsimd.dma_start(out=sb_f, in_=_part_bcast(s_fg_d.ap(), P))
            for ib in range(RB):
                nc.vector.tensor_scalar(m_cls[ib], sb_c,
                                        srow_c[:, ib:ib + 1], None, ALU.is_gt)
                nc.vector.tensor_scalar(m_fg[ib], sb_f,
                                        srow_f[:, ib:ib + 1], None, ALU.is_gt)

        # ================= phase C: attention megaloop (+D outputs) =======
        with tc.tile_pool(name="csb", bufs=1) as sb, \
             tc.tile_pool(name="cps", bufs=1, space="PSUM") as ps:
            for h in range(H):
                for ib in range(RB):
                    # --- logits, half-width psum tiles for pipelining ---
                    halves = {}
                    for (mat, qh, kh, msk) in (
                            ("c", qhatT_c, khatT_c, m_cls[ib]),
                            ("r", qhatT_r, khatT_r, m_fg[ib])):
                        for hf in range(2):
                            Sx = ps.tile([P, HQ], F32, tag="S", bufs=2,
                                         name=f"S{mat}{hf}_{h}_{ib}")
                            for fc in range(2):
                                fs = slice(hf * HQ + fc * FW,
                                           hf * HQ + (fc + 1) * FW)
                                nc.tensor.matmul(
                                    Sx[:, fc * FW:(fc + 1) * FW],
                                    qh[:, h, ib * P:(ib + 1) * P],
                                    kh[:, h, fs], start=True, stop=True)
                            # masked logits -> SBUF (psum in-place is slow:
                            # single DVE psum port serializes read+write)
                            Sm = sb.tile([P, HQ], F32, tag="Sm", bufs=2,
                                         name=f"Sm{mat}{hf}_{h}_{ib}")
                            nc.vector.tensor_tensor(
                                Sm, Sx, msk[:, hf * HQ:(hf + 1) * HQ],
                                op=ALU.mult)
                            halves[(mat, hf)] = Sm
                    # --- exp ---
                    Ec = sb.tile([P, N], BF16, tag="Ec", bufs=2,
                                 name=f"Ec{h}_{ib}")
                    Er = sb.tile([P, N], BF16, tag="Er", bufs=1,
                                 name=f"Er{h}_{ib}")
                    racc = {}
                    for mat, Ex, iq in (("c", Ec, invq_c[ib]),
                                        ("r", Er, invq_r[ib])):
                        for hf in range(2):
                            rx = sb.tile([P, 1], F32, tag=f"r{mat}{hf}",
                                         bufs=2, name=f"r{mat}{hf}_{h}_{ib}")
                            nc.scalar.activation(
                                Ex[:, hf * HQ:(hf + 1) * HQ],
                                halves[(mat, hf)], AF.Exp,
                                scale=iq[:, h:h + 1], accum_out=rx)
                            racc[(mat, hf)] = rx
                    # --- combine: attn = 0.5*Ec/rc + 0.5*Er/rr ---
                    rch = sb.tile([P, 1], F32, tag="rch", bufs=2,
                                  name=f"rch{h}_{ib}")
                    nc.vector.tensor_tensor(rch, racc[("c", 0)],
                                            racc[("c", 1)], op=ALU.add)
                    nc.vector.reciprocal(rch, rch)
                    nc.vector.tensor_scalar(rch, rch, 0.5, None, ALU.mult)
                    rrh = sb.tile([P, 1], F32, tag="rrh", bufs=2,
                                  name=f"rrh{h}_{ib}")
                    nc.vector.tensor_tensor(rrh, racc[("r", 0)],
                                            racc[("r", 1)], op=ALU.add)
                    nc.vector.reciprocal(rrh, rrh)
                    nc.vector.tensor_scalar(rrh, rrh, 0.5, None, ALU.mult)
                    at0 = sb.tile([P, N], BF16, tag="at0", bufs=1,
                                  name=f"at0{h}_{ib}")
                    nc.vector.tensor_scalar(at0, Er, rrh, None, ALU.mult)
                    t1 = sb.tile([P, N], BF16, tag="t1", bufs=1,
                                 name=f"t1{h}_{ib}")
                    nc.vector.tensor_scalar(t1, Ec, rch, None, ALU.mult)
                    attn = sb.tile([P, N], BF16, tag="attn", bufs=2,
                                   name=f"attn{h}_{ib}")
                    nc.vector.tensor_tensor(attn, t1, at0, op=ALU.add)
                    # --- sim accumulation (gpsimd, in place) ---
                    nc.gpsimd.tensor_tensor(out=sim_acc[ib], in0=sim_acc[ib],
                                            in1=attn, op=ALU.add)
                    # --- attn^T then PV ---
                    aT = sb.tile([P, NJB, P], BF16, tag="aTs", bufs=2,
                                 name=f"aT{h}_{ib}")
                    for half in range(2):
                        aTp = ps.tile([P, NJB // 2, P], BF16, tag="aT",
                                      bufs=2, name=f"aTp{half}_{h}_{ib}")
                        for j in range(NJB // 2):
                            jc = half * (NJB // 2) + j
                            nc.tensor.transpose(
                                aTp[:, j, :], attn[:, jc * P:(jc + 1) * P],
                                ident)
                        dst = aT[:, half * (NJB // 2):(half + 1) * (NJB // 2),
                                 :]
                        if half == 0:
                            nc.vector.tensor_copy(dst, aTp)
                        else:
                            nc.scalar.activation(dst, aTp, AF.Copy)
                    xps = ps.tile([P, D], F32, tag="xps", bufs=2,
                                  name=f"xps{h}_{ib}")
                    for jc in range(NJB):
                        nc.tensor.matmul(xps, aT[:, jc, :],
                                         v_nd[jc][:, h * P:(h + 1) * P],
                                         start=(jc == 0),
                                         stop=(jc == NJB - 1))
                    nc.vector.tensor_copy(x_acc[ib][:, h * P:(h + 1) * P],
                                          xps)

            # ---- phase D: outputs (same pools -> overlaps megaloop tail) --
            for ib in range(RB):
                nc.sync.dma_start(out=x_out_d.ap()[ib * P:(ib + 1) * P, 0:C],
                                  in_=x_acc[ib])
                esim = sb.tile([P, N], F32, tag="esim", name=f"esim{ib}")
                nc.scalar.activation(esim, sim_acc[ib], AF.Exp, scale=0.125)
                rs = sb.tile([P, 1], F32, tag="rs", name=f"rs{ib}")
                nc.vector.tensor_tensor(out=esim, in0=esim, in1=m_sim[ib],
                                        op=ALU.mult)
                nc.vector.tensor_reduce(out=rs, in_=esim, op=ALU.add,
                                        axis=mybir.AxisListType.X)
                rsi = sb.tile([P, 1], F32, tag="rsi", name=f"rsi{ib}")
                nc.vector.reciprocal(rsi, rs)
                nc.vector.tensor_scalar(esim, esim, rsi, None, ALU.mult)
                nc.sync.dma_start(out=sim_out_d.ap()[ib * P:(ib + 1) * P, :],
                                  in_=esim)

    nc.compile()
    return nc


_NC_CACHE = None


def _get_nc():
    global _NC_CACHE
    if _NC_CACHE is None:
        _NC_CACHE = build_nc()
    return _NC_CACHE


def make_in_maps(x_cls, x_reg, cls_score, fg_score, W_qkv_cls, W_qkv_reg):
    """Host-side sharding / layout prep (numpy only)."""
    x_cls = np.asarray(x_cls, np.float32)
    x_reg = np.asarray(x_reg, np.float32)
    cls_score = np.asarray(cls_score, np.float32)
    fg_score = np.asarray(fg_score, np.float32)
    W_qkv_cls = np.asarray(W_qkv_cls, np.float32)
    W_qkv_reg = np.asarray(W_qkv_reg, np.float32)

    xT_cls = np.ascontiguousarray(x_cls[0].T).astype(bf16)       # (C, N)
    xT_reg = np.ascontiguousarray(x_reg[0].T).astype(bf16)
    WT_cls = np.ascontiguousarray(W_qkv_cls.T).astype(bf16)      # (C, 3C)
    WT_reg = np.ascontiguousarray(W_qkv_reg[:2 * C].T).astype(bf16)

    in_maps = []
    for c in range(NCORES):
        rows = slice(c * R, (c + 1) * R)
        in_maps.append({
            "xT_cls": xT_cls, "xT_reg": xT_reg,
            "WT_cls": WT_cls, "WT_reg": WT_reg,
            "s_cls": cls_score, "s_fg": fg_score,
            "xTq_cls": np.ascontiguousarray(xT_cls[:, rows]),
            "xTq_reg": np.ascontiguousarray(xT_reg[:, rows]),
            "srow_cls_m": cls_score[rows] - np.float32(0.1),
            "srow_fg_m": fg_score[rows] - np.float32(0.1),
        })
    return in_maps


def assemble(results):
    x = np.concatenate([np.asarray(r["x_slice"]) for r in results], axis=0)
    sim = np.concatenate([np.asarray(r["sim_slice"]) for r in results],
                         axis=0)
    return x.reshape(1, N, 2 * C), sim


def kernel(x_cls, x_reg, cls_score, fg_score, W_qkv_cls, W_qkv_reg):
    nc = _get_nc()
    in_maps = make_in_maps(x_cls, x_reg, cls_score, fg_score,
                           W_qkv_cls, W_qkv_reg)
    res = bass_utils.run_bass_kernel_spmd(nc, in_maps,
                                          core_ids=list(range(NCORES)))
    return assemble(res.results)


if __name__ == "__main__":
    rng = np.random.default_rng(0)
    ins = {
        "x_cls": rng.standard_normal((1, N, C), dtype=np.float32),
        "x_reg": rng.standard_normal((1, N, C), dtype=np.float32),
        "cls_score": rng.random(N, dtype=np.float32),
        "fg_score": rng.random(N, dtype=np.float32),
        "W_qkv_cls": (rng.standard_normal((3 * C, C), dtype=np.float32) * 0.02),
        "W_qkv_reg": (rng.standard_normal((3 * C, C), dtype=np.float32) * 0.02),
    }
    x, sim = kernel(**ins)
    print("x:", x.shape, "sim:", sim.shape)


# revision 20
# speedup vs baseline: 1.0258x; 1.0258x over previous
"""Trainium2 Bass kernel for nn_Attention_msa (sparse masked-softmax attention).

Sharding: 8-way over query rows (each core owns 256 query rows for ALL 8
heads). The k/v projections (which need all 2048 keys) are computed
redundantly on every core — this avoids any collective (on-chip collectives
run at ~32 GB/s, so reducing a 16 MB [N,N] attention tensor would dominate).
The head-means of attn and attn_cls_raw (the sim outputs) then become purely
local reductions over the 8 heads each core already holds.

attn_cls_raw's head-mean uses the concat identity:
  mean_h vn_h[i].vn_h[j] = (1/H) * Vn[i].Vn[j],  Vn = concat_h vn_h  (N x C)
so it is a single C-contraction matmul per core instead of an H-reduction.

All matmuls run in bf16 (fp32 PSUM accumulate); masks, logits, exp and all
normalizers stay fp32. Emulated end-to-end numerics: max rel err ~2.5e-3.
"""
import sys

for _p in ("/opt/trn_rl_repo",):
    if _p not in sys.path:
        sys.path.insert(0, _p)

import numpy as np
import ml_dtypes

bf16 = ml_dtypes.bfloat16

import concourse.bacc as bacc
import concourse.bass as bass
import concourse.mybir as mybir
import concourse.tile as tile
from concourse import bass_utils
from concourse.masks import make_identity

F32 = mybir.dt.float32
BF16 = mybir.dt.bfloat16
AF = mybir.ActivationFunctionType
ALU = mybir.AluOpType

# Problem constants (hardcoded per harness contract).
N = 2048          # tokens
C = 1024          # channels
H = 8             # heads
D = 128           # head dim
P = 128           # SBUF partitions
NCORES = 8
R = N // NCORES   # query rows per core (256)
RB = R // P       # row blocks per core (2)
NJB = N // P      # key blocks (16)
NCB = C // P      # contraction chunks (8)
FW = 512          # matmul free-dim chunk
SH = FW // P      # head segments per free chunk (4)
HQ = N // 2       # half of N (psum S-tile width)
SCALE = 25.0


def _seg_bcast(ap_2d, seg, width):
    """(P, seg) tile -> broadcast AP (P, seg, width) with stride-0 inner dim."""
    return bass.AP(tensor=ap_2d.tensor, offset=ap_2d.offset,
                   ap=[list(ap_2d.ap[0]), [1, seg], [0, width]])


def _part_bcast(dram_ap, parts):
    """DRAM row (n,) -> AP (parts, n) broadcast across partitions."""
    return bass.AP(tensor=dram_ap.tensor, offset=dram_ap.offset,
                   ap=[[0, parts]] + [list(a) for a in dram_ap.ap])


def _col_view(dram_ap, ncols):
    """DRAM vector (ncols*128,) -> AP (128, ncols): tile[p, c] = v[c*128+p]."""
    return bass.AP(tensor=dram_ap.tensor, offset=dram_ap.offset,
                   ap=[[1, P], [P, ncols]])


def build_nc():
    import os
    from contextlib import ExitStack

    nc = bacc.Bacc("TRN2", target_bir_lowering=False, debug=False,
                   num_devices=NCORES)

    # ---- per-core DRAM I/O (identical program on every core; all
    # core-dependence flows through the input values) ----
    xT_cls_d = nc.dram_tensor("xT_cls", [C, N], BF16, kind="ExternalInput")
    xT_reg_d = nc.dram_tensor("xT_reg", [C, N], BF16, kind="ExternalInput")
    WT_cls_d = nc.dram_tensor("WT_cls", [C, 3 * C], BF16, kind="ExternalInput")
    WT_reg_d = nc.dram_tensor("WT_reg", [C, 2 * C], BF16, kind="ExternalInput")
    s_cls_d = nc.dram_tensor("s_cls", [N], F32, kind="ExternalInput")
    s_fg_d = nc.dram_tensor("s_fg", [N], F32, kind="ExternalInput")
    xTq_cls_d = nc.dram_tensor("xTq_cls", [C, R], BF16, kind="ExternalInput")
    xTq_reg_d = nc.dram_tensor("xTq_reg", [C, R], BF16, kind="ExternalInput")
    # srow_*_m = score[rows] - 0.1 (host-computed in fp32, exact)
    srow_cls_d = nc.dram_tensor("srow_cls_m", [R], F32, kind="ExternalInput")
    srow_fg_d = nc.dram_tensor("srow_fg_m", [R], F32, kind="ExternalInput")

    x_out_d = nc.dram_tensor("x_slice", [R, 2 * C], F32, kind="ExternalOutput")
    sim_out_d = nc.dram_tensor("sim_slice", [R, N], F32, kind="ExternalOutput")

    with tile.TileContext(nc) as tc, ExitStack() as ctx:
        pp = ctx.enter_context(tc.tile_pool(name="pp", bufs=1))

        # ---------- persistent tiles (alive to the end) ----------
        khatT_c = pp.tile([P, H, N], BF16, name="khatT_c")
        v_nd = [pp.tile([P, C], BF16, name=f"v_nd{jb}") for jb in range(NJB)]
        qhatT_c = pp.tile([P, H, R], BF16, name="qhatT_c")
        qhatT_r = pp.tile([P, H, R], BF16, name="qhatT_r")
        invq_c = [pp.tile([P, H], F32, name=f"invq_c{ib}") for ib in range(RB)]
        invq_r = [pp.tile([P, H], F32, name=f"invq_r{ib}") for ib in range(RB)]
        m_sim = [pp.tile([P, N], BF16, name=f"m_sim{ib}") for ib in range(RB)]
        sim_acc = [pp.tile([P, N], F32, name=f"sim_acc{ib}") for ib in range(RB)]
        x_acc = [pp.tile([P, C], F32, name=f"x_acc{ib}") for ib in range(RB)]
        ident = pp.tile([P, P], BF16, name="ident")
        scol_cls = pp.tile([P, NJB], F32, name="scol_cls")
        scol_fg = pp.tile([P, NJB], F32, name="scol_fg")
        srow_c = pp.tile([P, RB], F32, name="srow_c")
        srow_f = pp.tile([P, RB], F32, name="srow_f")

        # ================= shared projection machinery =================
        def kv_proj(xT_tiles, w_dram, w_col0, scol, dst_big, is_v,
                    sb, ps, wp, wtag):
            """All-token projection of one C-wide output group, streamed in
            512-wide output chunks (fc), 128-token blocks (jb)."""
            for fc in range(C // FW):
                w_tiles = []
                for cc in range(NCB):
                    wt = wp.tile([P, FW], BF16, tag=f"{wtag}{cc}",
                                 name=f"{wtag}_{cc}")
                    c0 = w_col0 + fc * FW
                    nc.sync.dma_start(
                        out=wt,
                        in_=w_dram.ap()[cc * P:(cc + 1) * P, c0:c0 + FW])
                    w_tiles.append(wt)
                for jb in range(NJB):
                    kps = ps.tile([P, FW], F32, tag="kps", name=f"kps{jb}")
                    for cc in range(NCB):
                        nc.tensor.matmul(
                            kps, xT_tiles[cc][:, jb * P:(jb + 1) * P],
                            w_tiles[cc], start=(cc == 0),
                            stop=(cc == NCB - 1))
                    if is_v:
                        nc.vector.tensor_copy(
                            v_nd[jb][:, fc * FW:(fc + 1) * FW], kps)
                    ksq = sb.tile([P, FW], BF16, tag="ksq", name=f"ksq{jb}")
                    nc.scalar.activation(ksq, kps, AF.Square)
                    ss = sb.tile([P, SH], F32, tag="ss", name=f"ss{jb}")
                    nc.vector.tensor_reduce(
                        out=ss, in_=ksq.rearrange("p (h e) -> p h e", h=SH),
                        op=ALU.add, axis=mybir.AxisListType.X)
                    nrm = sb.tile([P, SH], F32, tag="nrm", name=f"nrm{jb}")
                    nc.scalar.activation(nrm, ss, AF.Sqrt)
                    inv = sb.tile([P, SH], F32, tag="inv", name=f"inv{jb}")
                    nc.vector.reciprocal(inv, nrm)
                    if not is_v:
                        nc.vector.tensor_scalar(inv, inv, scol[:, jb:jb + 1],
                                                SCALE, ALU.mult, ALU.mult)
                    khat = sb.tile([P, FW], BF16, tag="khat", name=f"khat{jb}")
                    nc.vector.tensor_tensor(
                        out=khat.rearrange("p (h e) -> p h e", h=SH),
                        in0=kps.rearrange("p (h e) -> p h e", h=SH),
                        in1=_seg_bcast(inv, SH, P), op=ALU.mult)
                    tps = ps.tile([P, SH, P], BF16, tag="tps", name=f"tps{jb}")
                    for hh in range(SH):
                        nc.tensor.transpose(tps[:, hh, :],
                                            khat[:, hh * P:(hh + 1) * P],
                                            ident)
                    nc.vector.tensor_copy(
                        dst_big[:, fc * SH:(fc + 1) * SH,
                                jb * P:(jb + 1) * P],
                        tps)

        def q_proj(xTq_tiles, w_dram, w_col0, dst_big, invq, sb, ps, wp,
                   wtag, xori_cb=None, vnqT=None):
            for fc in range(C // FW):
                w_tiles = []
                for cc in range(NCB):
                    wt = wp.tile([P, FW], BF16, tag=f"{wtag}{cc}",
                                 name=f"{wtag}_{cc}")
                    c0 = w_col0 + fc * FW
                    nc.sync.dma_start(
                        out=wt,
                        in_=w_dram.ap()[cc * P:(cc + 1) * P, c0:c0 + FW])
                    w_tiles.append(wt)
                for ib in range(RB):
                    qps = ps.tile([P, FW], F32, tag="kps", name=f"qps{ib}")
                    for cc in range(NCB):
                        nc.tensor.matmul(
                            qps, xTq_tiles[cc][:, ib * P:(ib + 1) * P],
                            w_tiles[cc], start=(cc == 0),
                            stop=(cc == NCB - 1))
                    if xori_cb is not None:
                        xori_cb(qps, fc, ib)
                    qsq = sb.tile([P, FW], BF16, tag="ksq", name=f"qsq{ib}")
                    nc.scalar.activation(qsq, qps, AF.Square)
                    ss = sb.tile([P, SH], F32, tag="ss", name=f"qss{ib}")
                    nc.vector.tensor_reduce(
                        out=ss, in_=qsq.rearrange("p (h e) -> p h e", h=SH),
                        op=ALU.add, axis=mybir.AxisListType.X)
                    nrm = sb.tile([P, SH], F32, tag="nrm", name=f"qnrm{ib}")
                    nc.scalar.activation(nrm, ss, AF.Sqrt)
                    hs = slice(fc * SH, (fc + 1) * SH)
                    if vnqT is None:
                        nc.vector.reciprocal(invq[ib][:, hs], nrm)
                        qnd = sb.tile([P, FW], BF16, tag="khat",
                                      name=f"qnd{ib}")
                        nc.vector.tensor_copy(qnd, qps)
                    else:
                        inv = sb.tile([P, SH], F32, tag="inv", name=f"oi{ib}")
                        nc.vector.reciprocal(inv, nrm)
                        qnd = sb.tile([P, FW], BF16, tag="khat",
                                      name=f"ovn{ib}")
                        nc.vector.tensor_tensor(
                            out=qnd.rearrange("p (h e) -> p h e", h=SH),
                            in0=qps.rearrange("p (h e) -> p h e", h=SH),
                            in1=_seg_bcast(inv, SH, P), op=ALU.mult)
                    tps = ps.tile([P, SH, P], BF16, tag="tps",
                                  name=f"qtps{ib}")
                    for hh in range(SH):
                        nc.tensor.transpose(tps[:, hh, :],
                                            qnd[:, hh * P:(hh + 1) * P],
                                            ident)
                    dst = dst_big if vnqT is None else vnqT
                    nc.vector.tensor_copy(dst[:, hs, ib * P:(ib + 1) * P],
                                          tps)

        # ================= phase A: cls side =================
        # One shared pool set spans A1 (khat_cls), A2 (v/vn), A3 (q, x_ori,
        # vnq) so the scheduler can overlap sub-phases freely.
        with tc.tile_pool(name="vnTp", bufs=1) as vnTp, \
             tc.tile_pool(name="asb", bufs=2) as a_sb, \
             tc.tile_pool(name="aw", bufs=2) as a_wp:
            vnT_all = vnTp.tile([P, H, N], BF16, name="vnT_all")
            vnqT = vnTp.tile([P, H, R], BF16, name="vnqT")
            xTq = []
            for cc in range(NCB):
                t = vnTp.tile([P, R], BF16, name=f"xTqc{cc}")
                xTq.append(t)

            with tc.tile_pool(name="xtp", bufs=1) as xtp:
                # xT_cls load first (split in halves for smooth PE ramp)
                xT = []
                for cc in range(NCB):
                    t = xtp.tile([P, N], BF16, name=f"xTc{cc}")
                    nc.sync.dma_start(out=t[:, 0:HQ],
                                      in_=xT_cls_d.ap()[cc * P:(cc + 1) * P,
                                                        0:HQ])
                    nc.sync.dma_start(out=t[:, HQ:N],
                                      in_=xT_cls_d.ap()[cc * P:(cc + 1) * P,
                                                        HQ:N])
                    xT.append(t)
                for cc in range(NCB):
                    nc.sync.dma_start(
                        out=xTq[cc],
                        in_=xTq_cls_d.ap()[cc * P:(cc + 1) * P, :])
                make_identity(nc, ident)
                nc.sync.dma_start(out=scol_cls,
                                  in_=_col_view(s_cls_d.ap(), NJB))
                nc.sync.dma_start(out=scol_fg,
                                  in_=_col_view(s_fg_d.ap(), NJB))
                nc.sync.dma_start(out=srow_c,
                                  in_=_col_view(srow_cls_d.ap(), RB))
                nc.sync.dma_start(out=srow_f,
                                  in_=_col_view(srow_fg_d.ap(), RB))

                with tc.tile_pool(name="aps", bufs=4, space="PSUM") as a_ps:
                    # A1: khat_cls
                    kv_proj(xT, WT_cls_d, C, scol_cls, khatT_c, False,
                            a_sb, a_ps, a_wp, "w")
                    # A2: v (+ vn -> VnT)
                    kv_proj(xT, WT_cls_d, 2 * C, None, vnT_all, True,
                            a_sb, a_ps, a_wp, "w")
                    for ib in range(RB):
                        nc.gpsimd.memset(sim_acc[ib], 0.0)

                    # A3: q_cls, then x_ori + vnq (row-slice v)
                    q_proj(xTq, WT_cls_d, 0, qhatT_c, invq_c,
                           a_sb, a_ps, a_wp, "w")

                    def xori_cb(qps, fc, ib):
                        xo = a_sb.tile([P, FW], F32, tag="xori", bufs=1,
                                       name=f"xo{ib}")
                        nc.scalar.activation(xo, qps, AF.Copy)
                        nc.sync.dma_start(
                            out=x_out_d.ap()[ib * P:(ib + 1) * P,
                                             C + fc * FW:C + (fc + 1) * FW],
                            in_=xo)

                    q_proj(xTq, WT_cls_d, 2 * C, None, None,
                           a_sb, a_ps, a_wp, "w", xori_cb=xori_cb,
                           vnqT=vnqT)

            # A4: raw = Vn_rows @ Vn^T ; sim mask = (sum_h > 6.0)
            # (own psum pool; overlaps nothing psum-wise but xT is freed)
            with tc.tile_pool(name="a4ps", bufs=1, space="PSUM") as rps:
                for ib in range(RB):
                    raw = rps.tile([P, N], F32, tag="raw", name=f"raw{ib}")
                    for fc in range(4):
                        fs = slice(fc * FW, (fc + 1) * FW)
                        for h in range(H):
                            nc.tensor.matmul(
                                raw[:, fs],
                                vnqT[:, h, ib * P:(ib + 1) * P],
                                vnT_all[:, h, fs],
                                start=(h == 0), stop=(h == H - 1))
                    nc.vector.tensor_scalar(m_sim[ib], raw, 6.0, None,
                                            ALU.is_gt)

        # ================= phase B: reg side =================
        khp = ctx.enter_context(tc.tile_pool(name="khp", bufs=1))
        khatT_r = khp.tile([P, H, N], BF16, name="khatT_r")
        with tc.tile_pool(name="xtpr", bufs=1) as xtp2, \
             tc.tile_pool(name="bsb", bufs=2) as b_sb, \
             tc.tile_pool(name="bps", bufs=4, space="PSUM") as b_ps, \
             tc.tile_pool(name="bw", bufs=2) as b_wp:
            xTr = []
            for cc in range(NCB):
                t = xtp2.tile([P, N], BF16, name=f"xTr{cc}")
                nc.sync.dma_start(out=t[:, 0:HQ],
                                  in_=xT_reg_d.ap()[cc * P:(cc + 1) * P, 0:HQ])
                nc.sync.dma_start(out=t[:, HQ:N],
                                  in_=xT_reg_d.ap()[cc * P:(cc + 1) * P,
                                                    HQ:N])
                xTr.append(t)
            xTqr = []
            for cc in range(NCB):
                t = xtp2.tile([P, R], BF16, name=f"xTqr{cc}")
                nc.sync.dma_start(out=t,
                                  in_=xTq_reg_d.ap()[cc * P:(cc + 1) * P, :])
                xTqr.append(t)
            kv_proj(xTr, WT_reg_d, C, scol_fg, khatT_r, False,
                    b_sb, b_ps, b_wp, "w")
            q_proj(xTqr, WT_reg_d, 0, qhatT_r, invq_r,
                   b_sb, b_ps, b_wp, "w")

        # ---------------- masks (built just before the megaloop) ----------
        mp = ctx.enter_context(tc.tile_pool(name="mp", bufs=1))
        m_cls = [mp.tile([P, N], BF16, name=f"m_cls{ib}") for ib in range(RB)]
        m_fg = [mp.tile([P, N], BF16, name=f"m_fg{ib}") for ib in range(RB)]
        with tc.tile_pool(name="pre", bufs=1) as pre:
            sb_c = pre.tile([P, N], F32, name="sb_c")
            sb_f = pre.tile([P, N], F32, name="sb_f")
            nc.gpsimd.dma_start(out=sb_c, in_=_part_bcast(s_cls_d.ap(), P))
            nc.gpsimd.dma_start(out=sb_f, in_=_part_bcast(s_fg_d.ap(), P))
            for ib in range(RB):
                nc.vector.tensor_scalar(m_cls[ib], sb_c,
                                        srow_c[:, ib:ib + 1], None, ALU.is_gt)
                nc.vector.tensor_scalar(m_fg[ib], sb_f,
                                        srow_f[:, ib:ib + 1], None, ALU.is_gt)

        # ================= phase C: attention megaloop (+D outputs) =======
        with tc.tile_pool(name="csb", bufs=1) as sb, \
             tc.tile_pool(name="cps", bufs=1, space="PSUM") as ps:
            for h in range(H):
                for ib in range(RB):
                    # --- logits, half-width psum tiles for pipelining ---
                    halves = {}
                    for (mat, qh, kh, msk) in (
                            ("c", qhatT_c, khatT_c, m_cls[ib]),
                            ("r", qhatT_r, khatT_r, m_fg[ib])):
                        for hf in range(2):
                            Sx = ps.tile([P, HQ], F32, tag="S", bufs=2,
                                         name=f"S{mat}{hf}_{h}_{ib}")
                            for fc in range(2):
                                fs = slice(hf * HQ + fc * FW,
                                           hf * HQ + (fc + 1) * FW)
                                nc.tensor.matmul(
                                    Sx[:, fc * FW:(fc + 1) * FW],
                                    qh[:, h, ib * P:(ib + 1) * P],
                                    kh[:, h, fs], start=True, stop=True)
                            # masked logits -> SBUF (psum in-place is slow:
                            # single DVE psum port serializes read+write)
                            Sm = sb.tile([P, HQ], F32, tag="Sm", bufs=2,
                                         name=f"Sm{mat}{hf}_{h}_{ib}")
                            nc.vector.tensor_tensor(
                                Sm, Sx, msk[:, hf * HQ:(hf + 1) * HQ],
                                op=ALU.mult)
                            halves[(mat, hf)] = Sm
                    # --- exp ---
                    Ec = sb.tile([P, N], BF16, tag="Ec", bufs=2,
                                 name=f"Ec{h}_{ib}")
                    Er = sb.tile([P, N], BF16, tag="Er", bufs=1,
                                 name=f"Er{h}_{ib}")
                    racc = {}
                    for mat, Ex, iq in (("c", Ec, invq_c[ib]),
                                        ("r", Er, invq_r[ib])):
                        for hf in range(2):
                            rx = sb.tile([P, 1], F32, tag=f"r{mat}{hf}",
                                         bufs=2, name=f"r{mat}{hf}_{h}_{ib}")
                            nc.scalar.activation(
                                Ex[:, hf * HQ:(hf + 1) * HQ],
                                halves[(mat, hf)], AF.Exp,
                                scale=iq[:, h:h + 1], accum_out=rx)
                            racc[(mat, hf)] = rx
                    # --- combine: attn = 0.5*Ec/rc + 0.5*Er/rr ---
                    rch = sb.tile([P, 1], F32, tag="rch", bufs=2,
                                  name=f"rch{h}_{ib}")
                    nc.gpsimd.tensor_tensor(rch, racc[("c", 0)],
                                            racc[("c", 1)], op=ALU.add)
                    nc.vector.reciprocal(rch, rch)
                    nc.vector.tensor_scalar(rch, rch, 0.5, None, ALU.mult)
                    rrh = sb.tile([P, 1], F32, tag="rrh", bufs=2,
                                  name=f"rrh{h}_{ib}")
                    nc.gpsimd.tensor_tensor(rrh, racc[("r", 0)],
                                            racc[("r", 1)], op=ALU.add)
                    nc.vector.reciprocal(rrh, rrh)
                    nc.vector.tensor_scalar(rrh, rrh, 0.5, None, ALU.mult)
                    at0 = sb.tile([P, N], BF16, tag="at0", bufs=1,
                                  name=f"at0{h}_{ib}")
                    nc.vector.tensor_scalar(at0, Er, rrh, None, ALU.mult)
                    attn = sb.tile([P, N], BF16, tag="attn", bufs=2,
                                   name=f"attn{h}_{ib}")
                    nc.vector.scalar_tensor_tensor(
                        out=attn, in0=Ec, scalar=rch, in1=at0,
                        op0=ALU.mult, op1=ALU.add)
                    # --- sim accumulation (gpsimd, in place) ---
                    nc.gpsimd.tensor_tensor(out=sim_acc[ib], in0=sim_acc[ib],
                                            in1=attn, op=ALU.add)
                    # --- attn^T then PV ---
                    aT = sb.tile([P, NJB, P], BF16, tag="aTs", bufs=2,
                                 name=f"aT{h}_{ib}")
                    for half in range(2):
                        aTp = ps.tile([P, NJB // 2, P], BF16, tag="aT",
                                      bufs=2, name=f"aTp{half}_{h}_{ib}")
                        for j in range(NJB // 2):
                            jc = half * (NJB // 2) + j
                            nc.tensor.transpose(
                                aTp[:, j, :], attn[:, jc * P:(jc + 1) * P],
                                ident)
                        dst = aT[:, half * (NJB // 2):(half + 1) * (NJB // 2),
                                 :]
                        if half == 0:
                            nc.vector.tensor_copy(dst, aTp)
                        else:
                            nc.scalar.activation(dst, aTp, AF.Copy)
                    xps = ps.tile([P, D], F32, tag="xps", bufs=2,
                                  name=f"xps{h}_{ib}")
                    for jc in range(NJB):
                        nc.tensor.matmul(xps, aT[:, jc, :],
                                         v_nd[jc][:, h * P:(h + 1) * P],
                                         start=(jc == 0),
                                         stop=(jc == NJB - 1))
                    nc.vector.tensor_copy(x_acc[ib][:, h * P:(h + 1) * P],
                                          xps)

            # ---- phase D: outputs (same pools -> overlaps megaloop tail) --
            for ib in range(RB):
                nc.sync.dma_start(out=x_out_d.ap()[ib * P:(ib + 1) * P, 0:C],
                                  in_=x_acc[ib])
                esim = sb.tile([P, N], F32, tag="esim", name=f"esim{ib}")
                nc.scalar.activation(esim, sim_acc[ib], AF.Exp, scale=0.125)
                rs = sb.tile([P, 1], F32, tag="rs", name=f"rs{ib}")
                nc.vector.tensor_tensor(out=esim, in0=esim, in1=m_sim[ib],
                                        op=ALU.mult)
                nc.vector.tensor_reduce(out=rs, in_=esim, op=ALU.add,
                                        axis=mybir.AxisListType.X)
                rsi = sb.tile([P, 1], F32, tag="rsi", name=f"rsi{ib}")
                nc.vector.reciprocal(rsi, rs)
                nc.vector.tensor_scalar(esim, esim, rsi, None, ALU.mult)
                nc.sync.dma_start(out=sim_out_d.ap()[ib * P:(ib + 1) * P, :],
                                  in_=esim)

    nc.compile()
    return nc


_NC_CACHE = None


def _get_nc():
    global _NC_CACHE
    if _NC_CACHE is None:
        _NC_CACHE = build_nc()
    return _NC_CACHE


def make_in_maps(x_cls, x_reg, cls_score, fg_score, W_qkv_cls, W_qkv_reg):
    """Host-side sharding / layout prep (numpy only)."""
    x_cls = np.asarray(x_cls, np.float32)
    x_reg = np.asarray(x_reg, np.float32)
    cls_score = np.asarray(cls_score, np.float32)
    fg_score = np.asarray(fg_score, np.float32)
    W_qkv_cls = np.asarray(W_qkv_cls, np.float32)
    W_qkv_reg = np.asarray(W_qkv_reg, np.float32)

    xT_cls = np.ascontiguousarray(x_cls[0].T).astype(bf16)       # (C, N)
    xT_reg = np.ascontiguousarray(x_reg[0].T).astype(bf16)
    WT_cls = np.ascontiguousarray(W_qkv_cls.T).astype(bf16)      # (C, 3C)
    WT_reg = np.ascontiguousarray(W_qkv_reg[:2 * C].T).astype(bf16)

    in_maps = []
    for c in range(NCORES):
        rows = slice(c * R, (c + 1) * R)
        in_maps.append({
            "xT_cls": xT_cls, "xT_reg": xT_reg,
            "WT_cls": WT_cls, "WT_reg": WT_reg,
            "s_cls": cls_score, "s_fg": fg_score,
            "xTq_cls": np.ascontiguousarray(xT_cls[:, rows]),
            "xTq_reg": np.ascontiguousarray(xT_reg[:, rows]),
            "srow_cls_m": cls_score[rows] - np.float32(0.1),
            "srow_fg_m": fg_score[rows] - np.float32(0.1),
        })
    return in_maps


def assemble(results):
    x = np.concatenate([np.asarray(r["x_slice"]) for r in results], axis=0)
    sim = np.concatenate([np.asarray(r["sim_slice"]) for r in results],
                         axis=0)
    return x.reshape(1, N, 2 * C), sim


def kernel(x_cls, x_reg, cls_score, fg_score, W_qkv_cls, W_qkv_reg):
    nc = _get_nc()
    in_maps = make_in_maps(x_cls, x_reg, cls_score, fg_score,
                           W_qkv_cls, W_qkv_reg)
    res = bass_utils.run_bass_kernel_spmd(nc, in_maps,
                                          core_ids=list(range(NCORES)))
    return assemble(res.results)


if __name__ == "__main__":
    rng = np.random.default_rng(0)
    ins = {
        "x_cls": rng.standard_normal((1, N, C), dtype=np.float32),
        "x_reg": rng.standard_normal((1, N, C), dtype=np.float32),
        "cls_score": rng.random(N, dtype=np.float32),
        "fg_score": rng.random(N, dtype=np.float32),
        "W_qkv_cls": (rng.standard_normal((3 * C, C), dtype=np.float32) * 0.02),
        "W_qkv_reg": (rng.standard_normal((3 * C, C), dtype=np.float32) * 0.02),
    }
    x, sim = kernel(**ins)
    print("x:", x.shape, "sim:", sim.shape)
